# revision 21
# baseline (speedup 1.0000x reference)
"""Trainium2 Bass kernel for nn_DifferentiableParticleSystem (N=2048 GNN message passing).

Strategy (8 NeuronCores, shard receiver rows i):
  - Each core owns 256 receiver particles i; positions/velocities/mass replicated.
  - Edge features generated in [i-partition, j-free] plane layout, gathered per-i
    into [8, 2048] feature-major layout via DMA.
  - The 4-layer edge MLP runs on the PE as 2 block-diagonal fp16 matmuls per row:
      pass X = L1 (feat->h1) + L3 (h2->h3),  pass Y = L2 (h1->h2) + L4 (h3->z4, -z4)
    Biases are folded into the matmuls via a constant-ones rhs row, so both PSUM
    drains are pure relu and split freely across Act (activation) and DVE
    (tensor_scalar max).  The -W4 block gives z4n = relu(-z4-b4) so that
    tanh(z4) = tanh(z4p) - tanh(z4n) needs no sign-preserving passthrough.
  - h2 feeds pass X with a lag of TWO rows (yop_k -> rx slot k+2), removing the
    serial X->relu->Y->yop->X chain between consecutive rows.
  - z4p/z4n rows are scattered back to [i, 6*J] planes; tanh (Act), mask-mult and
    the j-reduction (GPSIMD mid-loop / DVE at tail) run in plane layout. The
    diagonal (j==i) term is removed by subtracting the on-device f_diag.
  - Particle integration epilogue runs per 128-i half in [i-partition, 3] layout.
"""

import numpy as np

N = 2048
NCORES = 8
NI_CORE = N // NCORES  # 256 receiver rows per core
DT_STEP = 0.016
GRAV_Y = -9.8

_RUNNER = None


# ----------------------------------------------------------------------------
# Bass program
# ----------------------------------------------------------------------------

def _build_nc(ni_core=NI_CORE):
    import concourse.bacc as bacc
    import concourse.mybir as mybir
    from concourse.tile import TileContext
    from concourse.mybir import AluOpType as op

    f32 = mybir.dt.float32
    f16 = mybir.dt.float16
    AF = mybir.ActivationFunctionType
    AX = mybir.AxisListType

    J = N              # sender dimension (full)
    S = 4              # j-streams
    FD = J // S        # 512 free columns per stream
    halves = [(h * 128, min(128, ni_core - h * 128))
              for h in range((ni_core + 127) // 128)]

    nc = bacc.Bacc("TRN2", target_bir_lowering=False)

    # ---- DRAM I/O ----
    posT = nc.dram_tensor("posT", [3, J], f32, kind="ExternalInput")
    velT = nc.dram_tensor("velT", [3, J], f32, kind="ExternalInput")
    massR = nc.dram_tensor("massR", [1, J], f32, kind="ExternalInput")
    pos_my = nc.dram_tensor("pos_my", [ni_core, 3], f32, kind="ExternalInput")
    vel_my = nc.dram_tensor("vel_my", [ni_core, 3], f32, kind="ExternalInput")
    mass_my = nc.dram_tensor("mass_my", [ni_core, 1], f32, kind="ExternalInput")
    ext_my = nc.dram_tensor("ext_my", [ni_core, 3], f32, kind="ExternalInput")
    ela_my = nc.dram_tensor("ela_my", [ni_core, 1], f32, kind="ExternalInput")
    fri_my = nc.dram_tensor("fri_my", [ni_core, 1], f32, kind="ExternalInput")
    W1d = nc.dram_tensor("W1", [8, 64], f32, kind="ExternalInput")
    b1d = nc.dram_tensor("b1", [1, 64], f32, kind="ExternalInput")
    W2d = nc.dram_tensor("W2", [64, 64], f32, kind="ExternalInput")
    b2d = nc.dram_tensor("b2", [1, 64], f32, kind="ExternalInput")
    W3d = nc.dram_tensor("W3", [64, 32], f32, kind="ExternalInput")
    b3d = nc.dram_tensor("b3", [1, 32], f32, kind="ExternalInput")
    W4d = nc.dram_tensor("W4", [32, 3], f32, kind="ExternalInput")
    b4d = nc.dram_tensor("b4", [1, 3], f32, kind="ExternalInput")
    out_pos = nc.dram_tensor("out_pos", [ni_core, 3], f32, kind="ExternalOutput")
    out_vel = nc.dram_tensor("out_vel", [ni_core, 3], f32, kind="ExternalOutput")

    # ---- persistent SBUF ----
    def sb(name, shape, dt):
        return nc.alloc_sbuf_tensor(name, shape, dt)

    pj = [sb(f"pj{d}", [128, J], f16) for d in range(3)]     # pos_j bcast planes
    vj = [sb(f"vj{d}", [128, J], f16) for d in range(3)]     # vel_j bcast planes
    imj = sb("imj", [128, J], f16)                           # 1/mass_j bcast
    FP = [sb(f"fp{h}", [128, 8 * J], f16) for h in range(len(halves))]
    ZP = [sb(f"zp{h}", [128, 6 * J], f16) for h in range(len(halves))]
    tA = sb("tA", [128, J], f16)    # tanh / feature-gen scratch
    tB = sb("tB", [128, J], f16)    # tanh / feature-gen scratch
    f32r = mybir.dt.float32r
    maskS = [sb(f"maskS{h}", [128, J], f16) for h in range(len(halves))]
    lhsTX = sb("lhsTX", [105, 96], f16)      # W3 @ rows 0-63; W1 @ 96-103; bias @ 104
    lhsTXl = sb("lhsTXl", [105, 96], f16)    # f16 residuals of the above
    lhsTY = sb("lhsTY", [97, 70], f32)       # W2 @ rows 0-63; W4/-W4 @ 64-95; bias @ 96
    RX = [sb(f"rx{r}", [105, J], f16) for r in range(3)]     # rhs ring for pass X
    RY = [sb(f"ry{r}", [97, J], f32) for r in range(2)]      # rhs ring for pass Y
    # (used as float32 by Act/DVE writers; bitcast to float32r at the matmul)
    negfd = sb("negfd", [128, 3], f32)                       # -(tanh p - tanh n) diag
    nf = [sb(f"nf{h}", [128, 3], f32) for h in range(len(halves))]

    Xps = nc.alloc_psum_tensor("xps", [96, J], f32)
    Yps = nc.alloc_psum_tensor("yps", [70, J], f32)

    with TileContext(nc) as tc:
        import contextlib
        with contextlib.ExitStack() as ctx:
            pool = ctx.enter_context(tc.tile_pool(name="misc", bufs=1))

            # ================= setup: weights =================
            # X pass: f16 hi + f16 residual (weight-precision recovery).
            # Y pass: float32r (full f32 weights at 1 cycle/row for free>=256).
            nc.vector.memset(lhsTX.ap()[:, :], 0.0)
            nc.vector.memset(lhsTXl.ap()[:, :], 0.0)

            def load_wx(dram_ap, rows, cols):
                nr, ncol = rows.stop - rows.start, cols.stop - cols.start
                w32 = pool.tile([64, 64], f32, tag="w32s")
                nc.sync.dma_start(w32[0:nr, 0:ncol], dram_ap)
                hi16 = pool.tile([64, 64], f16, tag="w16h")
                nc.vector.tensor_copy(hi16[0:nr, 0:ncol], w32[0:nr, 0:ncol])
                nc.vector.tensor_copy(lhsTX.ap()[rows, cols], hi16[0:nr, 0:ncol])
                res = pool.tile([64, 64], f32, tag="w32r")
                nc.vector.tensor_tensor(res[0:nr, 0:ncol], w32[0:nr, 0:ncol],
                                        hi16[0:nr, 0:ncol], op.subtract)
                nc.vector.tensor_copy(lhsTXl.ap()[rows, cols], res[0:nr, 0:ncol])

            load_wx(W1d[:, :], slice(96, 104), slice(0, 64))
            load_wx(W3d[:, :], slice(0, 64), slice(64, 96))
            # X bias row 104 = [b1 | b3] (partition 104 is not quadrant-aligned:
            # stage at partition 0 and DMA into place; residual row stays 0).
            bstgX = pool.tile([1, 96], f16, tag="bstgX")
            b32 = pool.tile([1, 96], f32, tag="b32stg")
            nc.sync.dma_start(b32[0:1, 0:64], b1d[0:1, 0:64])
            nc.sync.dma_start(b32[0:1, 64:96], b3d[0:1, 0:32])
            nc.vector.tensor_copy(bstgX[0:1, 0:96], b32[0:1, 0:96])
            nc.sync.dma_start(lhsTX.ap()[104:105, 0:96], bstgX[0:1, 0:96])
            # Y weights: f32 staging -> one f32r-rounded copy (the verifier
            # requires every producer feeding an fp32r matmul to round).
            ystg = pool.tile([97, 70], f32, tag="ystg")
            nc.vector.memset(ystg[:, :], 0.0)
            nc.sync.dma_start(ystg[0:64, 0:64], W2d[:, :])
            nc.sync.dma_start(ystg[64:96, 64:67], W4d[:, :])
            nc.vector.tensor_scalar(ystg[64:96, 67:70], ystg[64:96, 64:67],
                                    -1.0, None, op.mult)
            nc.sync.dma_start(ystg[96:97, 0:64], b2d[0:1, 0:64])
            nc.sync.dma_start(ystg[96:97, 64:67], b4d[0:1, 0:3])
            nc.vector.tensor_scalar(ystg[96:97, 67:70], ystg[96:97, 64:67],
                                    -1.0, None, op.mult)
            nc.vector.tensor_copy(lhsTY.ap()[:, :].bitcast(f32r),
                                  ystg[0:97, 0:70])

            # ================= setup: broadcast planes =================
            for d in range(3):
                r32 = pool.tile([1, J], f32, tag="rowstg32")
                r16 = pool.tile([1, J], f16, tag="rowstg16")
                nc.sync.dma_start(r32[:, :], posT[d:d + 1, :])
                nc.vector.tensor_copy(r16[:, :], r32[:, :])
                nc.gpsimd.partition_broadcast(pj[d].ap()[:, :], r16[0:1, :])
                r32v = pool.tile([1, J], f32, tag="rowstg32")
                r16v = pool.tile([1, J], f16, tag="rowstg16")
                nc.sync.dma_start(r32v[:, :], velT[d:d + 1, :])
                nc.vector.tensor_copy(r16v[:, :], r32v[:, :])
                nc.gpsimd.partition_broadcast(vj[d].ap()[:, :], r16v[0:1, :])
            stgm = pool.tile([1, J], f32, tag="rowstg32")
            stgm16 = pool.tile([1, J], f16, tag="rowstg16")
            nc.sync.dma_start(stgm[:, :], massR[:, :])
            nc.vector.reciprocal(stgm[:, :], stgm[:, :])
            nc.vector.tensor_copy(stgm16[:, :], stgm[:, :])
            nc.gpsimd.partition_broadcast(imj.ap()[:, :], stgm16[0:1, :])

            # ================= setup: ring constants =================
            onesrow = pool.tile([1, J], f16, tag="rowstg16")
            nc.vector.memset(onesrow[:, :], 1.0)
            for r in range(3):
                nc.vector.memset(RX[r].ap()[0:96, :], 0.0)        # h2/z4 region
                # ones (bias) row 104; rows 96:104 get overwritten by gathers
                nc.vector.memset(RX[r].ap()[96:105, :], 1.0)
            for r in range(2):
                nc.vector.tensor_copy(RY[r].ap()[96:97, :].bitcast(f32r),
                                      onesrow[0:1, :])            # ones (bias) row

            # ================= feature planes =================
            picol = {}

            def feat_gen(h, eng, sqrt_chunks):
                i0, Ph = halves[h]
                for d in range(3):
                    pc = pool.tile([128, 1], f32, tag=f"pic{h}{d}")
                    nc.sync.dma_start(pc[0:Ph, :], pos_my[i0:i0 + Ph, d:d + 1])
                    picol[(h, d)] = pc
                    eng.tensor_scalar(
                        FP[h].ap()[0:Ph, d * J:(d + 1) * J], pj[d].ap()[0:Ph, :],
                        pc[0:Ph, :], None, op.subtract)
                    vc = pool.tile([128, 1], f32, tag=f"vic{h}{d}")
                    nc.sync.dma_start(vc[0:Ph, :], vel_my[i0:i0 + Ph, d:d + 1])
                    eng.tensor_scalar(
                        FP[h].ap()[0:Ph, (3 + d) * J:(4 + d) * J], vj[d].ap()[0:Ph, :],
                        vc[0:Ph, :], None, op.subtract)
                # dist^2 -> scrA; mask before sqrt
                rel = lambda d: FP[h].ap()[0:Ph, d * J:(d + 1) * J]
                eng.tensor_tensor(tA.ap()[0:Ph, :], rel(0), rel(0), op.mult)
                eng.tensor_tensor(tB.ap()[0:Ph, :], rel(1), rel(1), op.mult)
                eng.tensor_tensor(tA.ap()[0:Ph, :], tA.ap()[0:Ph, :],
                                  tB.ap()[0:Ph, :], op.add)
                eng.tensor_tensor(tB.ap()[0:Ph, :], rel(2), rel(2), op.mult)
                eng.tensor_tensor(tA.ap()[0:Ph, :], tA.ap()[0:Ph, :],
                                  tB.ap()[0:Ph, :], op.add)
                eng.tensor_scalar(maskS[h].ap()[0:Ph, :], tA.ap()[0:Ph, :],
                                  1.0, None, op.is_lt)
                if sqrt_chunks:
                    feat_sqrt(h, 0, J)
                # mass ratio
                mc = pool.tile([128, 1], f32, tag=f"mic{h}")
                nc.sync.dma_start(mc[0:Ph, :], mass_my[i0:i0 + Ph, :])
                eng.tensor_scalar(FP[h].ap()[0:Ph, 7 * J:8 * J],
                                  imj.ap()[0:Ph, :], mc[0:Ph, :], None, op.mult)

            def feat_sqrt(h, c, nch=512):
                # dist = sqrt(dist^2) on Act (chunked to limit FIFO stalls)
                i0, Ph = halves[h]
                sl = slice(c * nch, c * nch + nch)
                nc.scalar.activation(FP[h].ap()[0:Ph, 6 * J + sl.start:
                                                6 * J + sl.stop],
                                     tA.ap()[0:Ph, sl], AF.Sqrt)

            feat_gen(0, nc.vector, 1)

            # ================= f_diag (on-device, same weights) =================
            def mm_x(out_ap, rhs_ap):
                nc.tensor.matmul(out_ap, lhsTX.ap()[:, :], rhs_ap, start=True,
                                 stop=False)
                nc.tensor.matmul(out_ap, lhsTXl.ap()[:, :], rhs_ap, start=False,
                                 stop=True)

            # (fp32r matmuls need an even free size -> width-2 chain, col 1 dead)
            fdx1 = pool.tile([105, 2], f16, tag="fdx1")
            nc.vector.memset(fdx1[0:96, :], 0.0)
            nc.vector.memset(fdx1[96:105, :], 1.0)    # ratio (103) + ones (104)
            nc.vector.memset(fdx1[96:103, :], 0.0)
            mm_x(Xps.ap()[0:96, 0:2], fdx1[0:105, :])
            fdy1 = pool.tile([97, 2], f32, tag="fdy1")
            nc.vector.tensor_scalar(fdy1[0:96, :].bitcast(f32r),
                                    Xps.ap()[0:96, 0:2], 0.0, None, op.max)
            nc.vector.tensor_copy(fdy1[96:97, :].bitcast(f32r), onesrow[0:1, 0:2])
            nc.tensor.matmul(Yps.ap()[0:70, 0:2],
                             lhsTY.ap()[:, :].bitcast(f32r),
                             fdy1[0:97, :].bitcast(f32r), start=True, stop=True)
            fdx2 = pool.tile([105, 2], f16, tag="fdx2")
            nc.vector.memset(fdx2[64:96, :], 0.0)
            nc.vector.memset(fdx2[96:105, :], 1.0)
            nc.vector.memset(fdx2[96:104, :], 0.0)
            nc.vector.tensor_scalar(fdx2[0:64, :], Yps.ap()[0:64, 0:2],
                                    0.0, None, op.max)
            mm_x(Xps.ap()[0:96, 0:2], fdx2[0:105, :])
            fdy2 = pool.tile([97, 2], f32, tag="fdy2")
            nc.vector.tensor_scalar(fdy2[0:96, :].bitcast(f32r),
                                    Xps.ap()[0:96, 0:2], 0.0, None, op.max)
            nc.vector.tensor_copy(fdy2[96:97, :].bitcast(f32r), onesrow[0:1, 0:2])
            nc.tensor.matmul(Yps.ap()[0:70, 0:2],
                             lhsTY.ap()[:, :].bitcast(f32r),
                             fdy2[0:97, :].bitcast(f32r), start=True, stop=True)
            fdpn = pool.tile([6, 1], f32, tag="fdpn")
            nc.vector.tensor_scalar(fdpn[0:6, :], Yps.ap()[64:70, 0:1],
                                    0.0, None, op.max)
            nc.scalar.activation(fdpn[0:6, :], fdpn[0:6, :], AF.Tanh)
            fdrow6 = pool.tile([1, 6], f32, tag="fdrow6")
            nc.gpsimd.dma_start(fdrow6[0:1, 0:6], fdpn[0:6, 0:1])
            fdrow = pool.tile([1, 3], f32, tag="fdrow")
            nc.vector.tensor_tensor(fdrow[0:1, 0:3], fdrow6[0:1, 3:6],
                                    fdrow6[0:1, 0:3], op.subtract)
            nc.gpsimd.partition_broadcast(negfd.ap()[:, :], fdrow[0:1, :])

            # ================= post-processing / epilogue =================
            # For the mid-loop half these are emitted as small chunks spread
            # over iterations so the in-order Act/DVE queues never stall the
            # pipeline; the final half runs them full-size (machine is idle).
            CH = 512
            nfp = pool.tile([128, 12], f32, tag="nfp0")

            def post_tanh_chunk(h, j):
                i0, Ph = halves[h]
                d, r = j // 8, j % 8
                pn, c = r // 4, r % 4
                dst = (tA if pn == 0 else tB).ap()
                plane = (3 * pn + d) * J
                nc.scalar.activation(
                    dst[0:Ph, c * CH:(c + 1) * CH],
                    ZP[h].ap()[0:Ph, plane + c * CH:plane + (c + 1) * CH],
                    AF.Tanh)

            def post_subm(h, d, eng):
                i0, Ph = halves[h]
                eng.tensor_tensor(tA.ap()[0:Ph, :], tA.ap()[0:Ph, :],
                                  tB.ap()[0:Ph, :], op.subtract)
                eng.tensor_tensor(tA.ap()[0:Ph, :], tA.ap()[0:Ph, :],
                                  maskS[h].ap()[0:Ph, :], op.mult)

            def post_reduce_chunk(h, d, c):
                i0, Ph = halves[h]
                nc.vector.reduce_sum(nfp[0:Ph, 4 * d + c:4 * d + c + 1],
                                     tA.ap()[0:Ph, c * CH:(c + 1) * CH], AX.X)

            def post_fin(h):
                i0, Ph = halves[h]
                for d in range(3):
                    nc.vector.reduce_sum(nf[h].ap()[0:Ph, d:d + 1],
                                         nfp[0:Ph, 4 * d:4 * d + 4], AX.X)
                nc.vector.tensor_tensor(nf[h].ap()[0:Ph, :], nf[h].ap()[0:Ph, :],
                                        negfd.ap()[0:Ph, :], op.add)
                nc.vector.tensor_scalar(nf[h].ap()[0:Ph, :], nf[h].ap()[0:Ph, :],
                                        10.0, None, op.mult)

            def do_post_tail(h):
                i0, Ph = halves[h]
                for d in range(3):
                    for pn in range(2):
                        dst = (tA if pn == 0 else tB).ap()
                        nc.scalar.activation(dst[0:Ph, :],
                                             ZP[h].ap()[0:Ph, (3 * pn + d) * J:
                                                        (3 * pn + d + 1) * J],
                                             AF.Tanh)
                    nc.vector.tensor_tensor(tA.ap()[0:Ph, :], tA.ap()[0:Ph, :],
                                            tB.ap()[0:Ph, :], op.subtract)
                    nc.vector.tensor_tensor(tA.ap()[0:Ph, :], tA.ap()[0:Ph, :],
                                            maskS[h].ap()[0:Ph, :], op.mult)
                    nc.vector.reduce_sum(nf[h].ap()[0:Ph, d:d + 1],
                                         tA.ap()[0:Ph, :], AX.X)
                nc.vector.tensor_tensor(nf[h].ap()[0:Ph, :], nf[h].ap()[0:Ph, :],
                                        negfd.ap()[0:Ph, :], op.add)
                nc.vector.tensor_scalar(nf[h].ap()[0:Ph, :], nf[h].ap()[0:Ph, :],
                                        10.0, None, op.mult)

            def post_epilogue(h):
                # ---- particle update epilogue ----
                i0, Ph = halves[h]
                ext = pool.tile([128, 3], f32, tag=f"ext{h}")
                nc.sync.dma_start(ext[0:Ph, :], ext_my[i0:i0 + Ph, :])
                pcol = pool.tile([128, 3], f32, tag=f"pc3{h}")
                nc.sync.dma_start(pcol[0:Ph, :], pos_my[i0:i0 + Ph, :])
                vcol = pool.tile([128, 3], f32, tag=f"vc3{h}")
                nc.sync.dma_start(vcol[0:Ph, :], vel_my[i0:i0 + Ph, :])
                mcol = pool.tile([128, 1], f32, tag=f"mm{h}")
                nc.sync.dma_start(mcol[0:Ph, :], mass_my[i0:i0 + Ph, :])
                ecol = pool.tile([128, 1], f32, tag=f"ee{h}")
                nc.sync.dma_start(ecol[0:Ph, :], ela_my[i0:i0 + Ph, :])
                fcol = pool.tile([128, 1], f32, tag=f"ff{h}")
                nc.sync.dma_start(fcol[0:Ph, :], fri_my[i0:i0 + Ph, :])

                frc = pool.tile([128, 3], f32, tag=f"frc{h}")
                nc.vector.tensor_tensor(frc[0:Ph, :], nf[h].ap()[0:Ph, :],
                                        ext[0:Ph, :], op.add)
                nc.vector.scalar_tensor_tensor(frc[0:Ph, 1:2], mcol[0:Ph, :],
                                               GRAV_Y, frc[0:Ph, 1:2],
                                               op.mult, op.add)
                imc = pool.tile([128, 1], f32, tag=f"imc{h}")
                nc.vector.reciprocal(imc[0:Ph, :], mcol[0:Ph, :])
                nc.vector.tensor_scalar(imc[0:Ph, :], imc[0:Ph, :], DT_STEP, None,
                                        op.mult)
                v1 = pool.tile([128, 3], f32, tag=f"v1{h}")
                nc.vector.scalar_tensor_tensor(v1[0:Ph, :], frc[0:Ph, :],
                                               imc[0:Ph, :], vcol[0:Ph, :],
                                               op.mult, op.add)
                sq = pool.tile([128, 3], f32, tag=f"sq{h}")
                nc.vector.tensor_tensor(sq[0:Ph, :], v1[0:Ph, :], v1[0:Ph, :],
                                        op.mult)
                spd = pool.tile([128, 1], f32, tag=f"spd{h}")
                nc.vector.reduce_sum(spd[0:Ph, :], sq[0:Ph, :], AX.X)
                nc.scalar.activation(spd[0:Ph, :], spd[0:Ph, :], AF.Sqrt)
                fm = pool.tile([128, 1], f32, tag=f"fm{h}")
                nc.vector.tensor_scalar(fm[0:Ph, :], spd[0:Ph, :], 0.1, None,
                                        op.is_gt)
                ffac = pool.tile([128, 1], f32, tag=f"ffac{h}")
                nc.vector.tensor_tensor(ffac[0:Ph, :], fcol[0:Ph, :], fm[0:Ph, :],
                                        op.mult)
                nc.vector.tensor_scalar(ffac[0:Ph, :], ffac[0:Ph, :], -DT_STEP, 1.0,
                                        op.mult, op.add)
                nc.vector.tensor_scalar(v1[0:Ph, :], v1[0:Ph, :], ffac[0:Ph, :],
                                        None, op.mult)
                p2 = pool.tile([128, 3], f32, tag=f"p2{h}")
                nc.vector.scalar_tensor_tensor(p2[0:Ph, :], v1[0:Ph, :], DT_STEP,
                                               pcol[0:Ph, :], op.mult, op.add)
                cm = pool.tile([128, 1], f32, tag=f"cm{h}")
                nc.vector.tensor_scalar(cm[0:Ph, :], p2[0:Ph, 1:2], 0.0, None,
                                        op.is_lt)
                ncm = pool.tile([128, 1], f32, tag=f"ncm{h}")
                nc.vector.tensor_scalar(ncm[0:Ph, :], cm[0:Ph, :], -1.0, 1.0,
                                        op.mult, op.add)
                nc.vector.tensor_tensor(p2[0:Ph, 1:2], p2[0:Ph, 1:2], ncm[0:Ph, :],
                                        op.mult)
                w1e = pool.tile([128, 1], f32, tag=f"w1e{h}")
                nc.vector.tensor_scalar(w1e[0:Ph, :], ecol[0:Ph, :], 1.0, None,
                                        op.add)
                nc.vector.tensor_tensor(w1e[0:Ph, :], cm[0:Ph, :], w1e[0:Ph, :],
                                        op.mult)
                nc.vector.tensor_scalar(w1e[0:Ph, :], w1e[0:Ph, :], -1.0, 1.0,
                                        op.mult, op.add)
                nc.vector.tensor_tensor(v1[0:Ph, 1:2], v1[0:Ph, 1:2], w1e[0:Ph, :],
                                        op.mult)
                nc.sync.dma_start(out_pos[i0:i0 + Ph, :], p2[0:Ph, :])
                nc.sync.dma_start(out_vel[i0:i0 + Ph, :], v1[0:Ph, :])

            # ================= main pipeline =================
            def gather(k):
                if k >= ni_core:
                    return
                h, p = (0, k) if k < 128 else (1, k - 128)
                src = FP[h].ap()[p:p + 1, :].rearrange("p (d j) -> p d j", d=8)
                nc.sync.dma_start(RX[k % 3].ap()[96:104, :], src)

            gather(0)
            gather(1)

            # ---- mid-loop burst schedule (half 0 post + half 1 features) ----
            sched = {}

            def at(k, fn):
                sched.setdefault(k, []).append(fn)

            if len(halves) > 1:
                at(4, lambda: feat_gen(1, nc.gpsimd, 0))
                for c in range(4):
                    at(40 + 2 * c, lambda c=c: feat_sqrt(1, c))
                yop_s3_act = set()
                for d in range(3):
                    t0 = 130 + 17 * d
                    for j in range(8):
                        at(t0 + j, lambda d=d, j=j: post_tanh_chunk(0, 8 * d + j))
                    at(t0 + 8, lambda d=d: post_subm(0, d, nc.gpsimd))
                    for c in range(4):
                        at(t0 + 13 + c, lambda d=d, c=c: post_reduce_chunk(0, d, c))
                        yop_s3_act.add(t0 + 13 + c)
                at(184, lambda: post_fin(0))
                at(186, lambda: post_epilogue(0))
            else:
                yop_s3_act = set()

            n_iter = ni_core + 2
            for k in range(n_iter):
                rx = RX[k % 3].ap()
                ry = RY[k % 2].ap()
                rxn = RX[(k + 2) % 3].ap()
                gather(k + 2)
                # ---- pass X (f16 hi + lo residual, accumulated in PSUM) ----
                for s in range(S):
                    sl = slice(s * FD, (s + 1) * FD)
                    nc.tensor.matmul(Xps.ap()[0:96, sl], lhsTX.ap()[:, :],
                                     rx[0:105, sl], start=True, stop=False)
                    nc.tensor.matmul(Xps.ap()[0:96, sl], lhsTXl.ap()[:, :],
                                     rx[0:105, sl], start=False, stop=True)
                # ---- relu drain -> f32r RY: Act s0,s1,s2 / DVE s3 ----
                for s in range(3):
                    sl = slice(s * FD, (s + 1) * FD)
                    nc.scalar.activation(ry[0:96, sl].bitcast(f32r),
                                         Xps.ap()[0:96, sl], AF.Relu)
                sl3 = slice(3 * FD, 4 * FD)
                nc.vector.tensor_scalar(ry[0:96, sl3].bitcast(f32r),
                                        Xps.ap()[0:96, sl3], 0.0, None, op.max)
                # ---- pass Y (single float32r matmul) ----
                for s in range(S):
                    sl = slice(s * FD, (s + 1) * FD)
                    nc.tensor.matmul(Yps.ap()[0:70, sl],
                                     lhsTY.ap()[:, :].bitcast(f32r),
                                     ry[0:97, sl].bitcast(f32r),
                                     start=True, stop=True)
                # ---- yop drain (h2 + z4p/z4n): Act s0 / DVE s1,s2,s3 ----
                sl0 = slice(0, FD)
                nc.scalar.activation(rxn[0:70, sl0], Yps.ap()[0:70, sl0], AF.Relu)
                for s in range(1, S):
                    sl = slice(s * FD, (s + 1) * FD)
                    eng = nc.scalar if (s == 3 and k in yop_s3_act) else nc.vector
                    if eng is nc.scalar:
                        nc.scalar.activation(rxn[0:70, sl], Yps.ap()[0:70, sl],
                                             AF.Relu)
                    else:
                        nc.vector.tensor_scalar(rxn[0:70, sl], Yps.ap()[0:70, sl],
                                                0.0, None, op.max)
                # ---- scatter z4p/z4n of row k-2 ----
                if k >= 2:
                    i = k - 2
                    h, p = (0, i) if i < 128 else (1, i - 128)
                    dst = ZP[h].ap()[p:p + 1, :].rearrange("p (d j) -> p d j", d=6)
                    nc.sync.dma_start(dst, rxn[64:70, :])
                # ---- scheduled bursts ----
                for fn in sched.get(k, ()):
                    fn()
            do_post_tail(len(halves) - 1)
            post_epilogue(len(halves) - 1)

    nc.compile()
    return nc


# ----------------------------------------------------------------------------
# Host-side runner (persistent jitted executable, modeled on
# concourse.bass2jax.run_bass_via_pjrt)
# ----------------------------------------------------------------------------

def _make_runner(ni_core=NI_CORE, n_cores=NCORES):
    import jax
    import numpy as _np
    from jax.sharding import Mesh, PartitionSpec
    try:
        from jax.experimental.shard_map import shard_map
    except Exception:
        from jax.shard_map import shard_map  # newer jax
    import concourse.mybir as mybir
    from concourse import bass2jax

    nc = _build_nc(ni_core)
    bass2jax.install_neuronx_cc_hook()

    pid_name = nc.partition_id_tensor.name if nc.partition_id_tensor else None
    in_names, out_names, out_avals = [], [], []
    for alloc in nc.m.functions[0].allocations:
        if not isinstance(alloc, mybir.MemoryLocationSet):
            continue
        name = alloc.memorylocations[0].name
        if alloc.kind == "ExternalInput":
            if name != pid_name:
                in_names.append(name)
        elif alloc.kind == "ExternalOutput":
            out_names.append(name)
            out_avals.append(jax.core.ShapedArray(
                tuple(alloc.tensor_shape), mybir.dt.np(alloc.dtype)))
    n_params = len(in_names)
    all_names = in_names + out_names
    if pid_name is not None:
        all_names = all_names + [pid_name]

    def _body(*args):
        operands = list(args)
        if pid_name is not None:
            operands.append(bass2jax.partition_id_tensor())
        outs = bass2jax._bass_exec_p.bind(
            *operands,
            out_avals=tuple(out_avals),
            in_names=tuple(all_names),
            out_names=tuple(out_names),
            lowering_input_output_aliases=(),
            sim_require_finite=True,
            sim_require_nnan=True,
            nc=nc,
        )
        return tuple(outs)

    devices = jax.devices()[:n_cores]
    mesh = Mesh(_np.asarray(devices), ("core",))
    donate = tuple(range(n_params, n_params + len(out_names)))
    sharded = jax.jit(
        shard_map(_body, mesh=mesh,
                  in_specs=(PartitionSpec("core"),) * (n_params + len(out_names)),
                  out_specs=(PartitionSpec("core"),) * len(out_names),
                  check_rep=False),
        donate_argnums=donate, keep_unused=True)

    def run(in_maps):
        per_core = [[_np.asarray(m[nm]) for nm in in_names] for m in in_maps]
        concat_in = [_np.concatenate([per_core[c][i] for c in range(n_cores)], 0)
                     for i in range(n_params)]
        concat_zeros = [
            _np.zeros((n_cores * a.shape[0], *a.shape[1:]), a.dtype)
            for a in out_avals]
        out_arrs = sharded(*concat_in, *concat_zeros)
        return [
            {nm: _np.asarray(out_arrs[i]).reshape(n_cores, *out_avals[i].shape)[c]
             for i, nm in enumerate(out_names)}
            for c in range(n_cores)
        ], out_arrs

    return run, in_names


def make_in_maps(inputs, ni_core=NI_CORE, n_cores=NCORES):
    f = lambda k: np.ascontiguousarray(np.asarray(inputs[k], dtype=np.float32))
    pos, vel, mass = f("positions"), f("velocities"), f("mass")
    ext, ela, fri = f("external_forces"), f("elasticity"), f("friction")
    base = {
        "posT": np.ascontiguousarray(pos.T),
        "velT": np.ascontiguousarray(vel.T),
        "massR": mass.reshape(1, N),
        "W1": f("W1"), "b1": f("b1").reshape(1, 64),
        "W2": f("W2"), "b2": f("b2").reshape(1, 64),
        "W3": f("W3"), "b3": f("b3").reshape(1, 32),
        "W4": f("W4"), "b4": f("b4").reshape(1, 3),
    }
    maps = []
    for c in range(n_cores):
        s = slice(c * ni_core, (c + 1) * ni_core)
        maps.append(dict(
            base,
            pos_my=np.ascontiguousarray(pos[s]),
            vel_my=np.ascontiguousarray(vel[s]),
            mass_my=np.ascontiguousarray(mass[s]).reshape(ni_core, 1),
            ext_my=np.ascontiguousarray(ext[s]),
            ela_my=np.ascontiguousarray(ela[s]).reshape(ni_core, 1),
            fri_my=np.ascontiguousarray(fri[s]).reshape(ni_core, 1),
        ))
    return maps


def get_runner():
    global _RUNNER
    if _RUNNER is None:
        _RUNNER = _make_runner()
    return _RUNNER


def kernel(**inputs):
    run, _ = get_runner()
    results, _ = run(make_in_maps(inputs))
    new_pos = np.concatenate([results[c]["out_pos"] for c in range(NCORES)], 0)
    new_vel = np.concatenate([results[c]["out_vel"] for c in range(NCORES)], 0)
    return new_pos, new_vel
